# revision 5
# baseline (speedup 1.0000x reference)
"""Trainium2 Bass kernel for MultiHeadEdgeAwareMessagePassing.

Math restructure (validated vs reference):
  logits[i,j,h] = s_q[i,h] + s_k[j,h] + w[i,j]*c1[h] + c0[h]   (valid j: w>0)
  alpha = softmax_j(logits) * w
s_q, c0 cancel in the softmax; bk's contribution to s_k cancels too. With
g[j,h] = exp(h[j]@a_k[h]), a_k[h] = Wk[h-block]^T u_k[h], v = h@Wv^T:
  msg[i,h,:] = Num_h[i,:] / Den_h[i]
  Num_h = W1^T (g_h*v_h)   (+ (W1^T g_h)*bv_h if bv != 0)
  Den_h = mask^T g_h + c1_h (W1^T g_h)
where mask=[w>0], W1=relu(w)  (exp(c1 w) ~= 1 + c1 w; |c1 w| << 1).

Sharding: destination rows i split across 8 cores (384 rows each). Each core
reads its relu(w)^T slice (fp8, host-pretiled for contiguous DMA), replicated
h^T (fp8) and small weights (bf16/f32). mask is recomputed on device from the
fp8 W1 via Sign on the scalar engine.

Schedule: software-pipelined by one chunk — the PE runs chunk ch-1's psA
matmuls while scalar/DVE produce chunk ch's rhs (v*g), so the PE stream stays
dense and HAM-warm. Epilogue is batched across the 3 i-subtiles: one psA PSUM
tile [128,3,512], residual-add via identity matmul, mean/var via scalar
accum_out, single fused output DMA.
"""

import numpy as np

N = 3072
D = 256
H = 4
DH = 64
DE = 8
NCORES = 8
ISLICE = N // NCORES       # 384
NSUB = ISLICE // 128       # 3
CJT = 4                    # j-tiles (of 128) per chunk
NCH = N // (128 * CJT)     # 6 chunks of 512 j
CHW = CJT * ISLICE         # 1536 wt cols per chunk

_cache = {}


def _build_bass(has_bv):
    import concourse.bass as bass
    import concourse.tile as tile
    from concourse import bacc, mybir
    from concourse.bass import ts

    dt = mybir.dt
    AF = mybir.ActivationFunctionType
    OP = mybir.AluOpType

    nc = bacc.Bacc("TRN2", target_bir_lowering=False, debug=False,
                   num_devices=NCORES)

    f8 = dt.float8e4
    bf = dt.bfloat16
    f32 = dt.float32

    # host-pretiled: wt[p, ch*1536 + jm*384 + ii] = relu(w)[i0+ii, (ch*4+jm)*128+p]
    wt_d = nc.dram_tensor("wt", [128, NCH * CHW], f8, kind="ExternalInput")
    # ht[p, a, j] = h[j, a*128+p]
    ht_d = nc.dram_tensor("ht", [128, 2, N], f8, kind="ExternalInput")
    # hs[p, s, d] = h[i0 + s*128 + p, d] + bo[d]
    hs_d = nc.dram_tensor("hs", [128, NSUB, D], bf, kind="ExternalInput")
    # su1[p, a, 0:256] = Wv^T block a ; su1[p, a, 256:260] = a_k block a
    su1_d = nc.dram_tensor("su1", [128, 2, 260], bf, kind="ExternalInput")
    # su2a: WoT 0:512 | identity 512:640 | gamma 640:896 | beta 896:1152
    su2a_d = nc.dram_tensor("su2a", [128, 1152], bf, kind="ExternalInput")
    # su2b f32: c1b 0:4 | (bv 4:260 if has_bv)
    su2b_cols = 4 + (256 if has_bv else 0)
    su2b_d = nc.dram_tensor("su2b", [128, su2b_cols], f32, kind="ExternalInput")
    # out[p, s, d] = result row (i0 + s*128 + p)
    out_d = nc.dram_tensor("out", [128, NSUB, D], f32, kind="ExternalOutput")

    with tile.TileContext(nc) as tc:
        with (
            tc.tile_pool(name="consts", bufs=1) as consts,
            tc.tile_pool(name="wtp", bufs=NCH) as wtp,
            tc.tile_pool(name="mskp", bufs=2) as mskp,
            tc.tile_pool(name="rhsp", bufs=4) as rhsp,
            tc.tile_pool(name="small", bufs=8) as small,
            tc.tile_pool(name="outp", bufs=3) as outp,
            tc.tile_pool(name="acc", bufs=1, space="PSUM") as accp,
            tc.tile_pool(name="pvk", bufs=2, space="PSUM") as pvk,
        ):
            # ---- setup consts (su1/ht on the scalar HWDGE queue, so their
            # issue overlaps with wt issue on the sync queue) ----
            su1 = consts.tile([128, 2, 260], bf, tag="su1")
            nc.scalar.dma_start(su1, su1_d.ap())
            eps_sb = consts.tile([128, 1], f32, tag="eps")
            nc.vector.memset(eps_sb, 1e-5)

            # persistent accumulator, one bank per i-subtile:
            # [:, s, 0:256] = W1.gV, 256:260 = W1.g, 260:264 = mask.g
            psA = accp.tile([128, NSUB, 512], f32, tag="A", name="psA")

            ht_sb = consts.tile([128, 2, N], f8, tag="ht")
            wt_tiles = [wtp.tile([128, CHW], f8, tag="wt", name=f"wt_{ch}")
                        for ch in range(NCH)]
            for ch in range(NCH):
                nc.scalar.dma_start(ht_sb[:, :, ts(ch, 128 * CJT)],
                                    ht_d.ap()[:, :, ts(ch, 128 * CJT)])
                nc.sync.dma_start(wt_tiles[ch], wt_d.ap()[:, ts(ch, CHW)])

            # ---------------- main loop (pipelined by one chunk) ---------
            rhs_tiles = {}

            def produce(ch):
                # mask for chunk ch (needed by consume(ch) later)
                msk = mskp.tile([128, CHW], f8, tag="msk", name=f"msk{ch}")
                nc.scalar.sign(msk, wt_tiles[ch])
                rhs_pair = []
                for hc in range(2):  # half-chunks of 2 j-tiles
                    ps_vk = pvk.tile([128, 2, 512], f32, tag="vk")
                    for jl in range(2):
                        jt = ch * CJT + hc * 2 + jl
                        for a in range(2):
                            nc.tensor.matmul(ps_vk[:, jl, 0:260],
                                             ht_sb[:, a, ts(jt, 128)],
                                             su1[:, a, :],
                                             start=(a == 0), stop=(a == 1))
                    rhs4 = rhsp.tile([128, 2, 260], bf, tag="rhs4",
                                     name=f"rhs{ch}_{hc}")
                    # g = exp(s_k) straight into the 4 tail cols of rhs4
                    nc.scalar.activation(rhs4[:, :, 256:260],
                                         ps_vk[:, :, 256:260], AF.Exp)
                    # rhs4[:, :, 0:256] = v * g (g broadcast over DH)
                    gv = rhs4[:, :, 256:260]
                    gb = bass.AP(tensor=gv.tensor, offset=gv.offset,
                                 ap=[gv.ap[0], gv.ap[1], gv.ap[2], [0, DH]])
                    nc.vector.tensor_tensor(
                        out=rhs4[:, :, 0:256].rearrange(
                            "p j (h d) -> p j h d", h=H),
                        in0=ps_vk[:, :, 0:256].rearrange(
                            "p j (h d) -> p j h d", h=H),
                        in1=gb, op=OP.mult)
                    rhs_pair.append(rhs4)
                rhs_tiles[ch] = (msk, rhs_pair)

            def consume(ch):
                wt4 = wt_tiles[ch]
                msk, rhs_pair = rhs_tiles.pop(ch)
                for hc in range(2):
                    rhs4 = rhs_pair[hc]
                    for jl in range(2):
                        off = (hc * 2 + jl) * ISLICE
                        st = (ch == 0 and hc == 0 and jl == 0)
                        sp = (ch == NCH - 1 and hc == 1 and jl == 1)
                        for s in range(NSUB):
                            sl = slice(off + s * 128, off + (s + 1) * 128)
                            nc.tensor.matmul(psA[:, s, 0:260], wt4[:, sl],
                                             rhs4[:, jl, :], start=st,
                                             stop=sp, skip_group_check=True)
                            nc.tensor.matmul(psA[:, s, 260:264], msk[:, sl],
                                             rhs4[:, jl, 256:260],
                                             start=st, stop=sp,
                                             skip_group_check=True)

            produce(0)
            for ch in range(NCH):
                if ch + 1 < NCH:
                    produce(ch + 1)
                consume(ch)

            # ---------------- epilogue consts (tail of sync queue) -------
            hs_sb = consts.tile([128, NSUB, D], bf, tag="hs")
            nc.sync.dma_start(hs_sb, hs_d.ap())
            su2a = consts.tile([128, 1152], bf, tag="su2a")
            nc.sync.dma_start(su2a, su2a_d.ap())
            su2b = consts.tile([128, su2b_cols], f32, tag="su2b")
            nc.sync.dma_start(su2b, su2b_d.ap())
            WoT_sb = su2a[:, 0:512].rearrange("p (a n) -> p a n", a=2)
            ident = su2a[:, 512:640]
            gam = su2a[:, 640:896]
            bet = su2a[:, 896:1152]
            c1b = su2b[:, 0:4]

            # ---------------- epilogue (batched across s) ----------------
            # den[p, s, h] = c1[h]*W1.g + mask.g ; rden = 1/den
            c1bb = bass.AP(tensor=c1b.tensor, offset=c1b.offset,
                           ap=[c1b.ap[0], [0, NSUB], c1b.ap[1]])
            den = small.tile([128, NSUB, H], f32, tag="den")
            nc.vector.tensor_tensor(out=den, in0=psA[:, :, 256:260],
                                    in1=c1bb, op=OP.mult)
            nc.vector.tensor_add(den, den, psA[:, :, 260:264])
            rden = small.tile([128, NSUB, H], f32, tag="rden")
            nc.vector.reciprocal(rden, den)

            # msg = Num * rden (rden broadcast over DH)
            num_in = psA[:, :, 0:256].rearrange("p s (h d) -> p s h d", h=H)
            if has_bv:
                gcol = psA[:, :, 256:260]
                gcb = bass.AP(tensor=gcol.tensor, offset=gcol.offset,
                              ap=[gcol.ap[0], gcol.ap[1], gcol.ap[2],
                                  [0, DH]])
                bvc = su2b[:, 4:260]
                bvb = bass.AP(tensor=bvc.tensor, offset=bvc.offset,
                              ap=[bvc.ap[0], [0, NSUB], [DH, H], [1, DH]])
                numf = outp.tile([128, NSUB, D], f32, tag="numf")
                nc.vector.tensor_tensor(
                    out=numf.rearrange("p s (h d) -> p s h d", h=H),
                    in0=gcb, in1=bvb, op=OP.mult)
                nc.vector.tensor_add(numf, numf, psA[:, :, 0:256])
                num_in = numf.rearrange("p s (h d) -> p s h d", h=H)
            msg = outp.tile([128, NSUB, D], bf, tag="msg")
            rdb = bass.AP(tensor=rden.tensor, offset=rden.offset,
                          ap=[rden.ap[0], rden.ap[1], rden.ap[2], [0, DH]])
            nc.vector.tensor_tensor(
                out=msg.rearrange("p s (h d) -> p s h d", h=H),
                in0=num_in, in1=rdb, op=OP.mult)

            # transpose msg per s; Wo matmul + residual add (identity mm)
            msgTs = []
            for s in range(NSUB):
                ps_t = pvk.tile([128, 2, 128], bf, tag="vk", name=f"pst{s}")
                for b in range(2):
                    nc.tensor.transpose(ps_t[:, b, :],
                                        msg[:, s, ts(b, 128)], ident)
                msgT = outp.tile([128, 2, 128], bf, tag="msgT",
                                 name=f"msgT{s}")
                nc.scalar.activation(msgT, ps_t, AF.Copy)
                msgTs.append(msgT)

            ps_o = accp.tile([128, NSUB, 512], f32, tag="A", name="pso")
            for s in range(NSUB):
                nc.tensor.matmul(ps_o[:, s, 0:256], msgTs[s][:, 0, :],
                                 WoT_sb[:, 0, :], start=True, stop=False)
                nc.tensor.matmul(ps_o[:, s, 0:256], msgTs[s][:, 1, :],
                                 WoT_sb[:, 1, :], start=False, stop=False)
                # x = msg@WoT + (h + bo): residual added on the PE
                nc.tensor.matmul(ps_o[:, s, 0:256], ident, hs_sb[:, s, :],
                                 start=False, stop=True)

            # layernorm: mean/var via scalar accum_out; y batched on DVE
            x_sb = outp.tile([128, NSUB, D], f32, tag="x")
            xs_j = outp.tile([128, NSUB, D], bf, tag="xsj")
            sm = small.tile([128, NSUB], f32, tag="sm")
            ssq = small.tile([128, NSUB], f32, tag="ssq")
            for s in range(NSUB):
                nc.scalar.activation(x_sb[:, s, :], ps_o[:, s, 0:256],
                                     AF.Copy, accum_out=sm[:, s:s + 1])
                nc.scalar.activation(xs_j[:, s, :], x_sb[:, s, :],
                                     AF.Square, accum_out=ssq[:, s:s + 1])
            mean = small.tile([128, NSUB], f32, tag="mean")
            nc.vector.tensor_scalar(mean, sm, 1.0 / D, None, op0=OP.mult)
            msq = small.tile([128, NSUB], f32, tag="msq")
            nc.vector.tensor_mul(msq, mean, mean)
            var = small.tile([128, NSUB], f32, tag="var")
            nc.vector.tensor_scalar(var, ssq, 1.0 / D, None, op0=OP.mult)
            nc.vector.tensor_tensor(out=var, in0=var, in1=msq,
                                    op=OP.subtract)
            sd = small.tile([128, NSUB], f32, tag="sd")
            nc.scalar.activation(sd, var, AF.Sqrt, bias=eps_sb)
            rstd = small.tile([128, NSUB], f32, tag="rstd")
            nc.vector.reciprocal(rstd, sd)

            y = outp.tile([128, NSUB, D], bf, tag="y")
            for s in range(NSUB):
                nc.vector.tensor_scalar(y[:, s, :], x_sb[:, s, :],
                                        mean[:, s:s + 1], rstd[:, s:s + 1],
                                        op0=OP.subtract, op1=OP.mult)
            gamb = bass.AP(tensor=gam.tensor, offset=gam.offset,
                           ap=[gam.ap[0], [0, NSUB], gam.ap[1]])
            betb = bass.AP(tensor=bet.tensor, offset=bet.offset,
                           ap=[bet.ap[0], [0, NSUB], bet.ap[1]])
            yg = outp.tile([128, NSUB, D], bf, tag="yg")
            nc.vector.tensor_tensor(out=yg, in0=y, in1=gamb, op=OP.mult)
            ot = outp.tile([128, NSUB, D], f32, tag="ot")
            nc.vector.tensor_tensor(out=ot, in0=yg, in1=betb, op=OP.add)
            nc.sync.dma_start(out_d.ap(), ot)

    nc.compile()
    return nc


def _make_in_maps(h, w, Wk, Wv, bv, We_w, u, Wo, bo, gamma, beta, **_unused):
    import ml_dtypes
    f = np.float32
    b16 = ml_dtypes.bfloat16
    e4 = ml_dtypes.float8_e4m3
    h = np.asarray(h, dtype=f)
    w = np.asarray(w, dtype=f)
    Wk = np.asarray(Wk, dtype=f)
    u = np.asarray(u, dtype=f)
    We_w = np.asarray(We_w, dtype=f)
    bv = np.asarray(bv, dtype=f)
    has_bv = bool(np.any(bv != 0))

    # ht[p, a, j] = h[j, a*128+p]
    ht = np.ascontiguousarray(
        h.T.reshape(2, 128, N).transpose(1, 0, 2)).astype(e4)

    # su1: Wv^T blocks + a_k blocks
    su1 = np.zeros((128, 2, 260), f)
    for a in range(2):
        su1[:, a, 0:256] = np.asarray(Wv, dtype=f)[:, a * 128:(a + 1) * 128].T
    ak = np.einsum('hdc,hd->ch', Wk.reshape(H, DH, D), u[:, DH:2 * DH])
    su1[:, 0, 256:260] = ak[0:128, :]
    su1[:, 1, 256:260] = ak[128:256, :]

    # su2a: WoT | identity | gamma | beta
    su2a = np.zeros((128, 1152), f)
    WoT = np.asarray(Wo, dtype=f).T
    su2a[:, 0:512] = WoT.reshape(2, 128, D).transpose(1, 0, 2).reshape(128, 512)
    su2a[:, 512:640] = np.eye(128, dtype=f)
    su2a[:, 640:896] = np.asarray(gamma, dtype=f)[None, :]
    su2a[:, 896:1152] = np.asarray(beta, dtype=f)[None, :]

    # su2b: c1 broadcast | (bv broadcast)
    c1 = np.einsum('hd,hd->h', We_w[:, 0].reshape(H, DE),
                   u[:, 2 * DH:2 * DH + DE])
    cols = 4 + (256 if has_bv else 0)
    su2b = np.zeros((128, cols), f)
    su2b[:, 0:4] = c1[None, :]
    if has_bv:
        su2b[:, 4:260] = bv[None, :]

    bo_f = np.asarray(bo, dtype=f)
    wT_relu = np.maximum(w.T, 0.0)

    common = {
        "ht": ht,
        "su1": su1.astype(b16),
        "su2a": su2a.astype(b16),
        "su2b": su2b,
    }
    in_maps = []
    for c in range(NCORES):
        sl = slice(c * ISLICE, (c + 1) * ISLICE)
        m = dict(common)
        # wt[p, ch, jm, ii] = relu(w)[i0+ii, (ch*4+jm)*128+p]
        wtc = wT_relu[:, sl].reshape(NCH, CJT, 128, ISLICE)
        m["wt"] = np.ascontiguousarray(
            wtc.transpose(2, 0, 1, 3).reshape(128, NCH * CHW)).astype(e4)
        m["hs"] = np.ascontiguousarray(
            (h[sl, :] + bo_f[None, :]).reshape(NSUB, 128, D).transpose(
                1, 0, 2)).astype(b16)
        in_maps.append(m)
    return in_maps, has_bv


def kernel(**inputs):
    from concourse.bass_utils import run_bass_kernel_spmd

    in_maps, has_bv = _make_in_maps(**inputs)
    key = "nc" if not has_bv else "nc_bv"
    if key not in _cache:
        _cache[key] = _build_bass(has_bv)
    nc = _cache[key]

    res = run_bass_kernel_spmd(nc, in_maps, core_ids=list(range(NCORES)))
    parts = [np.asarray(r["out"]).transpose(1, 0, 2).reshape(ISLICE, D)
             for r in res.results]
    out = np.concatenate(parts, axis=0)
    return np.ascontiguousarray(out, dtype=np.float32)
